# revision 14
# baseline (speedup 1.0000x reference)
"""DANNModel (MCLSTM + heads) Trainium2 kernel.

Strategy: data-parallel over batch across 8 NeuronCores (16 rows/core).
The 512-step recurrence runs fully on-device per core:
  gates = [h; x_t; 1] @ [U; W; bias]  (PE, fp16 in / fp32 PSUM, col order g|o|i|f, d separate)
  i,f,o = sigmoid, g = tanh (ACT); c1 = tanh(g)
  d = softmax(tanh(d_gates)) via exp(y) = 1/sigmoid(-y) - 1  (keeps one ACT table set)
  c_new = P*c1 + Q*c3 + R*c_old where [P|Q|R] = (e @ Kpqr)/Z  (rank-5 mixing matmul
  replaces the c2/c4 algebra inside the loop)
  h = sigmoid(o) * tanh(c_new); h transposed back via PE for the next step's lhsT.
Last step also emits c2/c4/d and fp32 casts. The tiny heads (0.02% of FLOPs,
full-batch batchnorm) run on host in fp32.
"""
import numpy as np

B, S, F, H = 128, 512, 64, 512
NCORES = 8
BC = B // NCORES  # 16
EPS = 1e-5

_cache = {}


# ---------------------------------------------------------------- IR post-pass
def _split_multi_waits(nc, max_waits=1):
    """This container's walrus rejects instructions with multiple sync waits
    (Tile's kernel-tail Drain aggregates one per live proc). Splice same-engine
    NoOps carrying one wait each in front of any such instruction."""
    import concourse.mybir as mybir
    ctr = [0]
    for f in nc.m.functions:
        for bb in f.blocks:
            new_list, changed = [], False
            for ins in bb.instructions:
                si = ins.sync_info
                if si is not None and si.on_wait and len(si.on_wait) > max_waits:
                    waits = list(si.on_wait)
                    for w in waits[:-max_waits]:
                        ctr[0] += 1
                        nop = mybir.InstNoOp(name=f"waitsplit_{id(bb)}_{ctr[0]}",
                                             ins=[], outs=[])
                        nop.engine = ins.engine
                        nop.sync_info = mybir.SyncInfo(on_wait=[w], on_update=[])
                        new_list.append(nop)
                    si.on_wait = waits[-max_waits:]
                    changed = True
                new_list.append(ins)
            if changed:
                bb.instructions = new_list


# ---------------------------------------------------------------- device build
def _build_nc(n_steps=S, ablate=()):
    import concourse.bass as bass
    import concourse.mybir as mybir
    import concourse.tile as tile
    from concourse.masks import make_identity
    from contextlib import ExitStack

    dt = mybir.dt
    Act = mybir.ActivationFunctionType
    Alu = mybir.AluOpType

    nc = bass.Bass("TRN2", target_bir_lowering=False, debug=False)

    xt_d = nc.dram_tensor("xt", (F + 1, S * BC), dt.float16, kind="ExternalInput")
    v5_d = nc.dram_tensor("v5", (128, 5 * 2048), dt.float16, kind="ExternalInput")
    vd_d = nc.dram_tensor("vd", (128, 5 * 8), dt.float16, kind="ExternalInput")
    kp_d = nc.dram_tensor("kpqr", (5, 1536), dt.float16, kind="ExternalInput")
    s1r_d = nc.dram_tensor("s1rep", (BC, H), dt.float32, kind="ExternalInput")
    s2r_d = nc.dram_tensor("s2rep", (BC, H), dt.float32, kind="ExternalInput")
    s1c_d = nc.dram_tensor("s1crep", (BC, H), dt.float32, kind="ExternalInput")
    s2c_d = nc.dram_tensor("s2crep", (BC, H), dt.float32, kind="ExternalInput")

    h_o = nc.dram_tensor("h_out", (BC, H), dt.float32, kind="ExternalOutput")
    c_os = [nc.dram_tensor(f"c{j}_out", (BC, H), dt.float32, kind="ExternalOutput")
            for j in range(1, 6)]
    d_o = nc.dram_tensor("d_out", (BC, 5), dt.float32, kind="ExternalOutput")

    with tile.TileContext(nc) as tc, ExitStack() as ctx:
        sb = ctx.enter_context(tc.tile_pool(name="sb", bufs=1))
        psG = ctx.enter_context(tc.tile_pool(name="psG", bufs=1, space="PSUM"))
        psP = ctx.enter_context(tc.tile_pool(name="psP", bufs=1, space="PSUM"))
        psS = ctx.enter_context(tc.tile_pool(name="psS", bufs=1, space="PSUM"))

        f16, f32 = dt.float16, dt.float32

        # constants / inputs resident in SBUF
        xt = sb.tile([F + 1, S * BC], f16)
        v5 = sb.tile([128, 5 * 2048], f16)
        vd = sb.tile([128, 5 * 8], f16)
        kp = sb.tile([5, 1536], f16)
        s1r = sb.tile([BC, H], f32)
        s2r = sb.tile([BC, H], f32)
        s1c = sb.tile([BC, H], f32)
        s2c = sb.tile([BC, H], f32)
        nc.sync.dma_start(xt[:, :], xt_d[:, :])
        nc.sync.dma_start(v5[:, :], v5_d[:, :])
        nc.sync.dma_start(vd[:, :], vd_d[:, :])
        nc.sync.dma_start(kp[:, :], kp_d[:, :])
        nc.sync.dma_start(s1r[:, :], s1r_d[:, :])
        nc.sync.dma_start(s2r[:, :], s2r_d[:, :])
        nc.sync.dma_start(s1c[:, :], s1c_d[:, :])
        nc.sync.dma_start(s2c[:, :], s2c_d[:, :])

        ident = sb.tile([16, 16], f16)
        make_identity(nc, ident[:, :])

        # state
        hT = sb.tile([128, 4 * BC], f16)   # h transposed, 4 chunks of [128, BC]
        cA = sb.tile([BC, H], f32)
        cB = sb.tile([BC, H], f32)
        nc.gpsimd.memset(hT[:, :], 0.0)
        nc.gpsimd.memset(cA[:, :], 0.0)

        # temps (persistent; steps are serial anyway)
        y5 = sb.tile([BC, 5], f32)
        sgy = sb.tile([BC, 5], f32)
        re5 = sb.tile([BC, 5], f32)
        zm5 = sb.tile([BC, 1], f32)
        zz = sb.tile([BC, 1], f32)
        rz = sb.tile([BC, 1], f32)
        stage = sb.tile([32, 32], f16)
        stageT = sb.tile([32, 32], f16)
        nc.gpsimd.memset(stage[:, :], 0.0)
        gS = sb.tile([BC, H], f16)
        c1S = sb.tile([BC, H], f16)
        sifS = sb.tile([BC, 2 * H], f16)
        oS2 = sb.tile([BC, H], f16)
        pqrS = sb.tile([BC, 1536], f16)
        t1 = sb.tile([BC, H], f32)
        t2 = sb.tile([BC, H], f16)
        c3S = sb.tile([BC, H], f16)
        m1 = sb.tile([BC, H], f16)
        m2 = sb.tile([BC, H], f16)
        m3 = sb.tile([BC, H], f16)
        s13 = sb.tile([BC, H], f16)
        cU = sb.tile([BC, H], f32)
        thS = sb.tile([BC, H], f16)
        hS = sb.tile([BC, H], f16)

        # last-step temps
        u1 = sb.tile([BC, H], f32)
        u2 = sb.tile([BC, H], f32)
        c2F = sb.tile([BC, H], f32)
        c4F = sb.tile([BC, H], f32)
        c1F = sb.tile([BC, H], f32)
        c3F = sb.tile([BC, H], f32)
        hF = sb.tile([BC, H], f32)
        dF = sb.tile([BC, 5], f32)

        # PSUM
        G = psG.tile([BC, 2048], f32)              # 4 banks: g|o|i|f
        PQR = psP.tile([BC, 1536], f32)            # 3 banks

        def zchunk(t, k):
            if k < 4:
                return hT[:, k * BC:(k + 1) * BC], 128
            return xt[:, t * BC:(t + 1) * BC], F + 1

        for t in range(n_steps):
            last = t == n_steps - 1
            cOld, cNew = (cA, cB) if t % 2 == 0 else (cB, cA)
            # one shared 1-bank PSUM slot per step: Gd early, hTps late
            Gd = psS.tile([BC, 8], f32, tag="small")
            hTps = psS.tile([128, 4 * BC], f16, tag="small")

            # ---- PE: d-gates (earliest; feeds the longest chain) ----
            for k in range(5):
                lhsT, rows = zchunk(t, k)
                nc.tensor.matmul(Gd[:, 0:5], lhsT, vd[0:rows, k * 8:k * 8 + 5],
                                 start=(k == 0), stop=(k == 4))

            # ---- d-path: y=tanh, e=exp(y)=1/sigmoid(-y)-1, Z, transpose ----
            if "dpath" not in ablate:
                nc.scalar.activation(y5[:, :], Gd[:, 0:5], Act.Tanh)
                nc.scalar.activation(sgy[:, :], y5[:, :], Act.Sigmoid, scale=-1.0)
                nc.vector.reciprocal(re5[:, :], sgy[:, :])
                # e = 1/sigmoid(-y) - 1 = exp(y); accum_out sums outputs -> Z
                nc.vector.tensor_scalar(stage[0:BC, 0:5], re5[:, :], -1.0, 0.0,
                                        Alu.add, Alu.add, accum_out=zm5[:, :])
                nc.vector.reciprocal(rz[:, :], zm5[:, :])
                nc.vector.transpose(stageT[:, :], stage[:, :])

            # ---- PE: main gate blocks (i|f|g|o), PQR right after block 0 ----
            for nb in range(4):
                for k in range(5):
                    lhsT, rows = zchunk(t, k)
                    nc.tensor.matmul(
                        G[:, nb * 512:(nb + 1) * 512], lhsT,
                        v5[0:rows, k * 2048 + nb * 512: k * 2048 + (nb + 1) * 512],
                        start=(k == 0), stop=(k == 4))
                if nb == 0 and "dpath" not in ablate:
                    for j in range(3):
                        nc.tensor.matmul(PQR[:, j * 512:(j + 1) * 512],
                                         stageT[0:5, 0:BC],
                                         kp[:, j * 512:(j + 1) * 512],
                                         start=True, stop=True)

            # ---- ACT: PQR evacuation + gate nonlinearities, earliest-first --
            iS = sifS[:, 0:512]
            fS = sifS[:, 512:1024]
            oS = oS2[:, :]
            if "actgates" not in ablate:
                nc.scalar.copy(pqrS[:, :], PQR[:, :])
                nc.scalar.activation(sifS[:, :], G[:, 0:1024], Act.Sigmoid)
                nc.scalar.activation(gS[:, :], G[:, 1024:1536], Act.Tanh)
                nc.scalar.activation(c1S[:, :], gS[:, :], Act.Tanh)
                nc.scalar.activation(oS2[:, :], G[:, 1536:2048], Act.Sigmoid)

            # ---- DVE: c3 = f*c + i*g ; c_new = (P*c1+Q*c3+R*c_old)/Z ----
            if "cnew" not in ablate:
                nc.vector.tensor_mul(m3[:, :], pqrS[:, 1024:1536], cOld[:, :])
            if "c3" not in ablate:
                nc.vector.tensor_mul(t1[:, :], fS, cOld[:, :])
                nc.vector.tensor_mul(t2[:, :], iS, gS[:, :])
                nc.vector.tensor_add(c3S[:, :], t1[:, :], t2[:, :])
            if "cnew" not in ablate:
                nc.vector.tensor_mul(m1[:, :], pqrS[:, 0:512], c1S[:, :])
                nc.vector.tensor_add(s13[:, :], m1[:, :], m3[:, :])
                nc.vector.tensor_mul(m2[:, :], pqrS[:, 512:1024], c3S[:, :])
                nc.vector.tensor_add(cU[:, :], s13[:, :], m2[:, :])
                nc.vector.tensor_scalar_mul(cNew[:, :], cU[:, :], rz[:, :])
            if "tail" not in ablate:
                nc.scalar.activation(thS[:, :], cU[:, :], Act.Tanh, scale=rz[:, :])
                nc.vector.tensor_mul(hS[:, :], oS, thS[:, :])

            # ---- PE: transpose h back for next step's lhsT ----
            if not last and "trans" not in ablate:
                for kc in range(4):
                    nc.tensor.transpose(hTps[:, kc * BC:(kc + 1) * BC],
                                        hS[:, kc * 128:(kc + 1) * 128],
                                        ident[:, :])
                nc.vector.tensor_copy(hT[:, :], hTps[:, :])

            if last:
                # c2 = s2*c3 + (1-s2)*c1 ; c4 = s1*c3 + (1-s1)*c5
                nc.vector.tensor_mul(u1[:, :], s2r[:, :], c3S[:, :])
                nc.vector.tensor_mul(u2[:, :], s2c[:, :], c1S[:, :])
                nc.vector.tensor_add(c2F[:, :], u1[:, :], u2[:, :])
                nc.vector.tensor_mul(u1[:, :], s1r[:, :], c3S[:, :])
                nc.vector.tensor_mul(u2[:, :], s1c[:, :], cOld[:, :])
                nc.vector.tensor_add(c4F[:, :], u1[:, :], u2[:, :])
                nc.vector.tensor_copy(c1F[:, :], c1S[:, :])
                nc.vector.tensor_copy(c3F[:, :], c3S[:, :])
                nc.vector.tensor_mul(hF[:, :], oS, thS[:, :])
                nc.vector.tensor_scalar_mul(dF[:, :], stage[0:BC, 0:5], rz[:, :])
                nc.sync.dma_start(h_o[:, :], hF[:, :])
                nc.sync.dma_start(c_os[0][:, :], c1F[:, :])
                nc.sync.dma_start(c_os[1][:, :], c2F[:, :])
                nc.sync.dma_start(c_os[2][:, :], c3F[:, :])
                nc.sync.dma_start(c_os[3][:, :], c4F[:, :])
                nc.sync.dma_start(c_os[4][:, :], cOld[:, :])
                nc.sync.dma_start(d_o[:, :], dF[:, :])

    _split_multi_waits(nc)
    return nc


# ---------------------------------------------------------------- host helpers
def _prep_consts(W, U, kernel1, bias, bias1):
    V = np.concatenate([U, W, bias[None, :]], 0).astype(np.float32)  # [577, 2053]
    Vg = V[:, 0:4 * H]          # i|f|g|o (natural order)  [577, 2048]
    Vd = V[:, 4 * H:4 * H + 5]  # [577, 5]
    v5 = np.zeros((128, 5 * 2048), np.float16)
    vd = np.zeros((128, 5 * 8), np.float16)
    for k in range(5):
        r0, r1 = k * 128, min((k + 1) * 128, 577)
        n = r1 - r0
        v5[0:n, k * 2048:(k + 1) * 2048] = Vg[r0:r1]
        vd[0:n, k * 8:k * 8 + 5] = Vd[r0:r1]

    kj = kernel1.reshape(5, H).astype(np.float32)
    s1 = bias1[:H].astype(np.float32)
    s2 = bias1[H:2 * H].astype(np.float32)
    KP = np.zeros((5, H), np.float32)
    KQ = np.zeros((5, H), np.float32)
    KR = np.zeros((5, H), np.float32)
    KP[0] = kj[0]
    KP[1] = kj[1] * (1 - s2)
    KQ[1] = kj[1] * s2
    KQ[2] = kj[2]
    KQ[3] = kj[3] * s1
    KR[3] = kj[3] * (1 - s1)
    KR[4] = kj[4]
    kpqr = np.concatenate([KP, KQ, KR], 1).astype(np.float16)

    reps = {
        "s1rep": np.repeat(s1[None, :], BC, 0).astype(np.float32),
        "s2rep": np.repeat(s2[None, :], BC, 0).astype(np.float32),
        "s1crep": np.repeat((1 - s1)[None, :], BC, 0).astype(np.float32),
        "s2crep": np.repeat((1 - s2)[None, :], BC, 0).astype(np.float32),
    }
    return v5, vd, kpqr, reps


def _heads(h, c1, c2, c3, c4, c5, d, p):
    def bn(x, g, b):
        m = x.mean(0)
        v = x.var(0)
        return (x - m) / np.sqrt(v + EPS) * g + b

    def relu(x):
        return np.maximum(x, 0.0)

    r = relu(h @ p["rw1"] + p["rb1"])
    r = relu(r @ p["rw2"] + p["rb2"])
    r = bn(r, p["rbng"], p["rbnb"])
    rul = r @ p["rw3"] + p["rb3"]

    def dom(feat):
        y = relu(feat @ p["dw1"] + p["db1"])
        y = bn(y, p["dbng"], p["dbnb"])
        z = y @ p["dw2"] + p["db2"]
        mz = z.max(1, keepdims=True)
        return z - (mz + np.log(np.exp(z - mz).sum(1, keepdims=True)))

    return (rul.astype(np.float32), dom(c1), dom(c2), dom(c3), dom(c4), dom(c5),
            d.astype(np.float32))


def _run_device(x, v5, vd, kpqr, reps, n_steps=S, trace=False):
    from concourse.bass_utils import run_bass_kernel_spmd
    key = n_steps
    if key not in _cache:
        _cache[key] = _build_nc(n_steps)
    nc = _cache[key]
    in_maps = []
    for c in range(NCORES):
        xc = x[c * BC:(c + 1) * BC]  # [BC, S, F]
        xtc = np.zeros((F + 1, S * BC), np.float16)
        xtc[0:F] = np.transpose(xc, (2, 1, 0)).reshape(F, S * BC)
        xtc[F] = 1.0
        in_maps.append({"xt": xtc, "v5": v5, "vd": vd, "kpqr": kpqr, **reps})
    res = run_bass_kernel_spmd(nc, in_maps, core_ids=list(range(NCORES)),
                               trace=trace)
    cat = lambda n: np.concatenate([res.results[c][n] for c in range(NCORES)], 0)
    out = (cat("h_out"), cat("c1_out"), cat("c2_out"), cat("c3_out"),
           cat("c4_out"), cat("c5_out"), cat("d_out"))
    return out, res


def kernel(**inputs):
    p = {k: np.asarray(v) for k, v in inputs.items()}
    x = p["input_data"].astype(np.float32)
    v5, vd, kpqr, reps = _prep_consts(p["W"].astype(np.float32),
                                      p["U"].astype(np.float32),
                                      p["kernel1"].astype(np.float32),
                                      p["bias"].astype(np.float32),
                                      p["bias1"].astype(np.float32))
    (h, c1, c2, c3, c4, c5, d), _ = _run_device(x, v5, vd, kpqr, reps)
    pf = {k: p[k].astype(np.float32) for k in
          ("rw1", "rb1", "rw2", "rb2", "rbng", "rbnb", "rw3", "rb3",
           "dw1", "db1", "dbng", "dbnb", "dw2", "db2")}
    return _heads(h, c1, c2, c3, c4, c5, d, pf)
